# revision 16
# baseline (speedup 1.0000x reference)
"""Causal self-attention TRN2 Bass kernel (bf16, software-pipelined).

Sharding: 8 cores = 4 batches x 2 head-groups. Core c handles batch c//2 and
heads (c%2)*8 .. (c%2)*8+8 (of 16). Each core computes its heads' attention
and a partial output projection; the host sums the two partials per batch and
adds b_out.

Design notes:
  - all matmul operands bf16 (FWL weight loads overlap the stream; f32r
    self-loading matmuls serialize a ~180ns weight load per matmul)
  - DRAM inputs are host-packed into small fine-grained tiles so the first
    S matmul only waits on ~1.5MB (xT is token-chunk-major, weights per-ft)
  - single pool scope, one long instruction stream: QK/V/out projections are
    emitted as deadline-scheduled "filler" half-chains interleaved into the
    attention cadence, so the PE never idles
  - causal mask applied by zeroing exp(S) tiles (affine_select on gpsimd/
    vector), not by adding -inf into PSUM: keeps DVE off the S->exp path
  - softmax denominator via the ones-column of V (row 64 of the PV PSUM);
    normalization split: PSUM drain (copies) immediate, recip/broadcast/mult
    deferred into later steps as filler DVE/gpsimd work
  - warmup matmuls ramp the PE p-state while input DMAs stream

Layouts on chip (per core):
  XTC   4 x [128, 8x512] bf16  x[b].T token-chunk-major: chunk c, d-chunk l
  WQF/WKF 4 x [128, 8x128] bf16 per-ft Q/K weights; WV 2 x [128, 4x512]
  WOH   2 x [128, 4x512] bf16
  QKT   8 x [128, 2048] bf16  Q^T (0..3) / K^T (4..7) features x tokens
  V     16 x [128, 520] bf16  tokens x (8 heads x (64 vals + ones col))
  e     [128, 1024] bf16      exp(S^T) per k-tile, both heads
  AOT   4 x [128, 2048] bf16  normalized attention out (features x tokens)
  y     [2048, 1024] bf16     partial output projection
"""
import sys

sys.path.insert(0, "/opt/trn_rl_repo")

import numpy as np
import ml_dtypes

D_MODEL = 1024
N_HEADS = 16
B = 4
T = 2048
HD = 64
N_CORES = 8
NH_LOC = N_HEADS // 2  # heads per core
FQ = NH_LOC * HD  # 512 local features

_prog_cache = {}


def build_program(tok=T, debug_dumps=False):
    """Build the single-core SPMD Bass program. tok must be a multiple of 512."""
    import concourse.mybir as mybir
    import concourse.tile as tile
    from concourse import bacc

    f32 = mybir.dt.float32
    bf16 = mybir.dt.bfloat16
    P = 128
    QC = 512  # q-chunk width
    KC = D_MODEL // P  # 8 d-model chunks
    TT = tok // P  # token tiles
    NJ = tok // QC  # q-chunks
    NDC = FQ // P  # 4 feature chunks

    nc = bacc.Bacc("TRN2", target_bir_lowering=False, debug=False, num_devices=N_CORES)

    fp8 = mybir.dt.float8e4
    DR = mybir.MatmulPerfMode.DoubleRow

    # fine-grained DRAM inputs (host-packed); one tensor per DMA piece so
    # tile-granular dependencies stay small.  The QKV projections run in
    # fp8e4 DoubleRow mode (2 k-chunks per instruction at 2x rate) with a
    # hi/lo residual split: a = hi + lo with hi = fp8(a); the dropped
    # lo*lo term is ~0.1%.  Weights are host-scaled x16 so their residual
    # stays in fp8 normal range (exp scale and wo absorb the 1/256, 1/16).
    xTh = [
        nc.dram_tensor(f"xTh{c}", [P, KC * QC], fp8, kind="ExternalInput")
        for c in range(NJ)
    ]
    xTl = [
        nc.dram_tensor(f"xTl{c}", [P, KC * QC], fp8, kind="ExternalInput")
        for c in range(NJ)
    ]
    wqf = [
        nc.dram_tensor(f"wq{ft}{t}", [P, KC * P], fp8, kind="ExternalInput")
        for ft in range(NDC)
        for t in ("h", "l")
    ]
    wkf = [
        nc.dram_tensor(f"wk{ft}{t}", [P, KC * P], fp8, kind="ExternalInput")
        for ft in range(NDC)
        for t in ("h", "l")
    ]
    wvt = [
        nc.dram_tensor(f"wv{t}", [P, KC * FQ], fp8, kind="ExternalInput")
        for t in ("h", "l")
    ]
    woh = [
        nc.dram_tensor(f"wo{h}", [P, NDC * QC], bf16, kind="ExternalInput")
        for h in range(2)
    ]
    y = nc.dram_tensor("y", [tok, D_MODEL], bf16, kind="ExternalOutput")

    with tile.TileContext(nc) as tc:
        with (
            tc.tile_pool(name="wp", bufs=1) as wp,
            tc.tile_pool(name="xtp", bufs=1) as xtp,
            tc.tile_pool(name="qktp", bufs=1) as qktp,
            tc.tile_pool(name="vp", bufs=1) as vp,
            tc.tile_pool(name="aotp", bufs=1) as aotp,
            tc.tile_pool(name="ep", bufs=3) as ep,
            tc.tile_pool(name="ystp", bufs=4) as ystp,
            tc.tile_pool(name="mvp", bufs=1) as mvp,
            tc.tile_pool(name="nrm", bufs=6) as nrmp,
            tc.tile_pool(name="nrs", bufs=6) as nrsp,
            tc.tile_pool(name="big", bufs=2, space="PSUM") as bigp,   # 4 banks
            tc.tile_pool(name="pvp", bufs=2, space="PSUM") as pvp,    # 2 banks
            tc.tile_pool(name="prj", bufs=2, space="PSUM") as prjp,   # 2 banks
        ):
            XTH = [wp.tile([P, KC * QC], fp8, tag=f"xth{c}", name=f"xth{c}") for c in range(NJ)]
            XTL = [wp.tile([P, KC * QC], fp8, tag=f"xtl{c}", name=f"xtl{c}") for c in range(NJ)]
            WQF = [wp.tile([P, KC * P], fp8, tag=f"wqf{i}", name=f"wqf{i}") for i in range(2 * NDC)]
            WKF = [wp.tile([P, KC * P], fp8, tag=f"wkf{i}", name=f"wkf{i}") for i in range(2 * NDC)]
            WVT = [wp.tile([P, KC * FQ], fp8, tag=f"wvt{t}", name=f"wvt{t}") for t in range(2)]
            WOH = [wp.tile([P, NDC * QC], bf16, tag=f"woh{h}", name=f"woh{h}") for h in range(2)]
            QKT = [qktp.tile([P, tok], bf16, tag=f"qkt{i}", name=f"qkt{i}") for i in range(8)]
            V = [vp.tile([P, NH_LOC * (HD + 1)], bf16, tag=f"v{i}", name=f"v{i}") for i in range(TT)]
            AOT = [aotp.tile([P, tok], bf16, tag=f"aot{d}", name=f"aot{d}") for d in range(NDC)]

            # DoubleRow pair views: pair pi covers d-chunks (2pi, 2pi+1)
            def xt_pair(c, t, pi):  # [128, 2, 512] moving pair
                src = XTH[c] if t == 0 else XTL[c]
                return src[:, 2 * pi * QC : (2 * pi + 2) * QC].rearrange(
                    "p (two n) -> p two n", two=2
                )

            def xt_tok_pair(c, t, pi, s):  # [128, 2, 128] stationary pair
                src = XTH[c] if t == 0 else XTL[c]
                return src[:].rearrange("p (l n) -> p l n", n=QC)[
                    :, 2 * pi : 2 * pi + 2, s * P : (s + 1) * P
                ]

            def wq_pair(ft, t, pi):  # [128, 2, 128]
                return WQF[2 * ft + t][:, 2 * pi * P : (2 * pi + 2) * P].rearrange(
                    "p (two n) -> p two n", two=2
                )

            def wk_pair(ft, t, pi):
                return WKF[2 * ft + t][:, 2 * pi * P : (2 * pi + 2) * P].rearrange(
                    "p (two n) -> p two n", two=2
                )

            def wv_pair(t, pi):  # [128, 2, 512]
                return WVT[t][:, 2 * pi * FQ : (2 * pi + 2) * FQ].rearrange(
                    "p (two n) -> p two n", two=2
                )

            def wo_view(h, d):
                return WOH[h][:, d * QC : (d + 1) * QC]

            # ---------------- input DMAs ----------------
            # three issue queues in parallel, DMA-bandwidth-ordered: the
            # first-S critical set (xtc0 hi+lo + wq_ft0 + wk_ft0, ~1.5MB)
            # leads, split so no queue serialises more than its share.
            nc.gpsimd.dma_start(out=XTH[0][:], in_=xTh[0][:])
            nc.sync.dma_start(out=XTL[0][:], in_=xTl[0][:])
            nc.scalar.dma_start(out=WQF[0][:], in_=wqf[0][:])
            nc.scalar.dma_start(out=WKF[0][:], in_=wkf[0][:])
            nc.scalar.dma_start(out=WQF[1][:], in_=wqf[1][:])
            nc.scalar.dma_start(out=WKF[1][:], in_=wkf[1][:])
            nc.gpsimd.dma_start(out=WVT[0][:], in_=wvt[0][:])
            nc.sync.dma_start(out=WVT[1][:], in_=wvt[1][:])
            nc.gpsimd.dma_start(out=XTH[1][:], in_=xTh[1][:])
            nc.sync.dma_start(out=XTL[1][:], in_=xTl[1][:])
            nc.scalar.dma_start(out=WQF[2][:], in_=wqf[2][:])
            nc.scalar.dma_start(out=WKF[2][:], in_=wkf[2][:])
            nc.scalar.dma_start(out=WQF[3][:], in_=wqf[3][:])
            nc.scalar.dma_start(out=WKF[3][:], in_=wkf[3][:])
            nc.gpsimd.dma_start(out=XTH[2][:], in_=xTh[2][:])
            nc.sync.dma_start(out=XTL[2][:], in_=xTl[2][:])
            nc.gpsimd.dma_start(out=XTH[3][:], in_=xTh[3][:])
            nc.sync.dma_start(out=XTL[3][:], in_=xTl[3][:])
            nc.scalar.dma_start(out=WQF[4][:], in_=wqf[4][:])
            nc.scalar.dma_start(out=WKF[4][:], in_=wkf[4][:])
            nc.scalar.dma_start(out=WQF[5][:], in_=wqf[5][:])
            nc.scalar.dma_start(out=WKF[5][:], in_=wkf[5][:])
            nc.scalar.dma_start(out=WQF[6][:], in_=wqf[6][:])
            nc.scalar.dma_start(out=WKF[6][:], in_=wkf[6][:])
            nc.scalar.dma_start(out=WQF[7][:], in_=wqf[7][:])
            nc.scalar.dma_start(out=WKF[7][:], in_=wkf[7][:])
            nc.sync.dma_start(out=WOH[0][:], in_=woh[0][:])
            nc.sync.dma_start(out=WOH[1][:], in_=woh[1][:])

            # warm the exp table while input DMAs stream
            warm = mvp.tile([1, 8], f32, tag="warm", name="warm")
            nc.vector.memset(warm[:], 0.0)
            nc.scalar.activation(warm[:], warm[:], mybir.ActivationFunctionType.Exp)

            # ones columns of V (value cols are written by the projection
            # eviction; only col 64 of each head group needs initialising).
            # On the vector queue: gpsimd is busy issuing DMAs and must be
            # free early for the first diagonal ezeros.
            for tt in range(TT):
                vv = V[tt][:].rearrange("p (u c) -> p u c", c=HD + 1)
                nc.vector.memset(vv[:, :, HD : HD + 1], 1.0)

            # PE p-state warmup: dummy matmuls with no DMA deps keep the PE
            # "continuously executing" so real matmuls start at full clock
            dwa = mvp.tile([P, P], bf16, tag="dwa", name="dwa")
            dwb = mvp.tile([P, QC], bf16, tag="dwb", name="dwb")
            nc.vector.memset(dwa[:], 0.0)
            nc.vector.memset(dwb[:], 0.0)
            pwarm = prjp.tile([P, QC], f32, tag="prj", name="pwarm")
            for _ in range(16):
                nc.tensor.matmul(pwarm[:, :256], dwa[:], dwb[:, :256], start=True, stop=True)

            # ---------------- filler chains (emitted in halves) ----------------
            open_chains = {}

            # fp8 DoubleRow 3-term residual chains: hi*hi + lo*hi + hi*lo,
            # 12 DR matmuls (4 pairs x 3 terms) into one PSUM chain, split
            # in two halves for the filler scheduler.
            QK_TERMS = [(0, 0, pi) for pi in range(4)] + [(1, 0, pi) for pi in range(4)] + [(0, 1, pi) for pi in range(4)]

            def qk_half(ft, c, part):
                """QKT[ft][:, c-chunk] = (w-slice)^T @ XT; fp8 DR, 2 halves."""
                wpair = wq_pair if ft < 4 else wk_pair
                fi = ft % 4
                key = ("qk", ft, c)
                if part == 0:
                    open_chains[key] = prjp.tile([P, QC], f32, tag="prj", name=f"pqk{ft}_{c}")
                p = open_chains[key]
                terms = QK_TERMS[6 * part : 6 * part + 6]
                for n, (tw, tx, pi) in enumerate(terms):
                    nc.tensor.matmul(
                        p[:],
                        wpair(fi, tw, pi),
                        xt_pair(c, tx, pi),
                        start=(part == 0 and n == 0),
                        stop=(part == 1 and n == 5),
                        perf_mode=DR,
                    )
                if part == 1:
                    del open_chains[key]
                    nc.vector.tensor_copy(QKT[ft][:, c * QC : (c + 1) * QC], p[:])

            def v_half(tt, part):
                """V[tt] value cols = XT-slice^T @ WV; fp8 DR, 2 halves."""
                key = ("v", tt)
                if part == 0:
                    open_chains[key] = prjp.tile([P, FQ], f32, tag="prj", name=f"pv{tt}")
                p = open_chains[key]
                c, s = tt // 4, tt % 4
                terms = QK_TERMS[6 * part : 6 * part + 6]
                for n, (tw, tx, pi) in enumerate(terms):
                    nc.tensor.matmul(
                        p[:],
                        xt_tok_pair(c, tx, pi, s),
                        wv_pair(tw, pi),
                        start=(part == 0 and n == 0),
                        stop=(part == 1 and n == 5),
                        perf_mode=DR,
                    )
                if part == 1:
                    del open_chains[key]
                    vdst = V[tt][:].rearrange("p (u c) -> p u c", c=HD + 1)[:, :, 0:HD]
                    vsrc = p[:].rearrange("p (u c) -> p u c", c=HD)
                    nc.vector.tensor_copy(vdst, vsrc)

            def out_chain(tt, h, evict=None):
                """y[tt-tile, h-half] = AOT-slice^T @ WO, 4 MMs + copy + DMA."""
                p = prjp.tile([P, QC], f32, tag="prj", name=f"py{tt}_{h}")
                for d in range(NDC):
                    nc.tensor.matmul(
                        p[:],
                        AOT[d][:, tt * P : (tt + 1) * P],
                        wo_view(h, d),
                        start=(d == 0),
                        stop=(d == NDC - 1),
                    )
                ysb = ystp.tile([P, QC], bf16, tag="y")
                if evict is None:
                    nc.vector.tensor_copy(ysb[:], p[:])
                else:
                    evict(ysb[:], p[:])
                nc.sync.dma_start(
                    out=y[tt * P : (tt + 1) * P, h * QC : (h + 1) * QC],
                    in_=ysb[:],
                )

            # per-hp filler schedules: {step: [unit, ...]}.  hp0's schedule is
            # deadline-driven (chunk c is read by attention from j=c onward at
            # step 4*c*(c+1)/2...; V[tt] is read by PV at the k-step for tile
            # tt of each j >= tt//4).
            sched = {hp: {} for hp in range(4)}

            def put(hp, step, fn):
                sched[hp].setdefault(step, []).append(fn)

            # hp0, deadline-driven: QK c1 at steps 0-3 (read from step 4),
            # V4-7 at 4-7 (read 8-11), QK c2 at 8-11 (read 12), V8-11 at
            # 12-15 (read 16-19), QK c3 at 16-19 (read 24), V12-15 at 20-23
            # (read 28-31)
            for k, (ft, c) in enumerate([(0, 1), (4, 1)]):
                for part in (0, 1):
                    put(0, 2 * k + part, lambda ft=ft, c=c, part=part: qk_half(ft, c, part))
            for tt in (4, 5, 6, 7):
                for part in (0, 1):
                    put(0, tt, lambda tt=tt, part=part: v_half(tt, part))
            for k, (ft, c) in enumerate([(0, 2), (4, 2)]):
                for part in (0, 1):
                    put(0, 8 + 2 * k + part, lambda ft=ft, c=c, part=part: qk_half(ft, c, part))
            for tt in (8, 9, 10, 11):
                for part in (0, 1):
                    put(0, 4 + tt, lambda tt=tt, part=part: v_half(tt, part))
            for k, (ft, c) in enumerate([(0, 3), (4, 3)]):
                for part in (0, 1):
                    put(0, 16 + 2 * k + part, lambda ft=ft, c=c, part=part: qk_half(ft, c, part))
            for tt in (12, 13, 14, 15):
                for part in (0, 1):
                    put(0, 8 + tt, lambda tt=tt, part=part: v_half(tt, part))
            # QK(hp+1) spread over each hp's steps (hp0's start after its
            # deadline units; hp1/hp2 evenly — ~1 unit per 2.5 steps keeps
            # the per-step PE load under the exp cadence)
            for hp in range(3):
                base = 24 if hp == 0 else 0
                units = []
                for ft in (hp + 1, 4 + hp + 1):
                    for c in range(NJ):
                        for part in (0, 1):
                            units.append(
                                lambda ft=ft, c=c, part=part: qk_half(ft, c, part)
                            )
                # finish by ~80% of the hp's steps so the next head-pair's
                # first S never waits on the last chain's eviction
                span = max(len(units), int(0.8 * (40 - base)))
                for k, fn in enumerate(units):
                    put(hp, base + (k * span) // len(units), fn)

            # ---------------- pre-attention work ----------------
            # half-chain interleave: the QK c0 chains (gating the first S)
            # complete as early as the DMA pieces allow, V chains fill in
            qk_half(0, 0, 0)
            qk_half(4, 0, 0)
            qk_half(0, 0, 1)
            qk_half(4, 0, 1)
            for tt in range(4):
                for part in (0, 1):
                    v_half(tt, part)

            # ---------------- attention + interleaved fillers ----------------
            out_ready = []  # out-proj chains unlocked so far
            pending_fin = []  # deferred normalization finishers
            for hp in range(4):
                step = 0
                for j in range(NJ):
                    nkt = 4 * j + 4
                    pv = {
                        u: pvp.tile([HD + 1, QC], f32, tag="pv", name=f"pv{u}")
                        for u in (0, 1)
                    }
                    etiles = {}

                    def emit_s_exp(i, j=j, etiles=etiles):
                        """S-pair + exp (+ causal zeroing) for k-tile i."""
                        s_ = i - 4 * j
                        w0 = 128 * s_ if s_ >= 0 else 0
                        st = bigp.tile([P, 2 * QC], f32, tag="big", name="st")
                        for u in (0, 1):
                            rs = slice(64 * u, 64 * u + 64)
                            nc.tensor.matmul(
                                st[:, u * QC + w0 : (u + 1) * QC],
                                QKT[4 + hp][rs, i * P : (i + 1) * P],
                                QKT[hp][rs, j * QC + w0 : (j + 1) * QC],
                                start=True,
                                stop=True,
                            )
                        win3 = st[:].rearrange("p (h q) -> p h q", h=2)
                        e = ep.tile([P, 2 * QC], bf16, tag="e", name="e")
                        nc.scalar.activation(
                            e[:].rearrange("p (h q) -> p h q", h=2)[:, :, w0:QC],
                            win3[:, :, w0:QC],
                            mybir.ActivationFunctionType.Exp,
                            scale=0.125 / 256.0,  # 1/sqrt(64) and the 16x16
                            # host scaling of wq/wk
                        )
                        if s_ >= 0:
                            # causal mask: zero exp(S) where q < k in the
                            # diagonal 128x128 block (iota = q_hat - p >= 0
                            # keeps; else fill 0).  gpsimd/vector split.
                            for u, eng in ((0, nc.gpsimd), (1, nc.gpsimd)):
                                ev = e[:, u * QC + w0 : u * QC + w0 + P]
                                eng.affine_select(
                                    out=ev,
                                    in_=ev,
                                    compare_op=mybir.AluOpType.is_ge,
                                    fill=0.0,
                                    base=0,
                                    pattern=[[1, P]],
                                    channel_multiplier=-1,
                                )
                        etiles[i] = e

                    emit_s_exp(0)
                    if nkt > 1:
                        emit_s_exp(1)
                    for i in range(nkt):
                        # software pipeline, lookahead 2: issue S/exp two
                        # k-tiles ahead of this PV so the PE's in-order queue
                        # always has an S-pair between consecutive PVs and the
                        # ACT stream gets a full step of slack
                        if i + 2 < nkt:
                            emit_s_exp(i + 2)
                        s_ = i - 4 * j
                        w0 = 128 * s_ if s_ >= 0 else 0
                        e = etiles.pop(i)
                        for u in (0, 1):
                            hloc = 2 * hp + u
                            nc.tensor.matmul(
                                pv[u][:, w0:QC],
                                V[i][:, hloc * (HD + 1) : (hloc + 1) * (HD + 1)],
                                e[:, u * QC + w0 : (u + 1) * QC],
                                start=(i == 0),
                                stop=(i == nkt - 1),
                            )
                        # deferred norm finishers go first (they unlock AOT
                        # for out chains), on non-diagonal steps only so the
                        # gpsimd broadcast never delays a diagonal ezero
                        if pending_fin and s_ < 0:
                            pending_fin.pop(0)()
                        for fn in sched[hp].pop(step, ()):
                            fn()
                        step += 1
                        # out-proj chains become fillers once unlocked; in the
                        # last head-pair keep 4 in reserve so the PE has work
                        # to chew while the final chunk's normalization runs
                        if out_ready and len(out_ready) > 4:
                            out_chain(*out_ready.pop(0))
                    # normalization, split in two: drain PSUM now (copies of
                    # the accumulator rows and the denominator row), defer
                    # recip/broadcast/mult into later steps.  For the very
                    # last chunk, u1's copies go to the (now exp-free) scalar
                    # engine so the two PSUM drains run in parallel.
                    last = hp == 3 and j == NJ - 1
                    fins = []
                    for u in (0, 1):
                        sa = nrmp.tile([HD, QC], f32, tag="sa", name="sa")
                        sd = nrsp.tile([1, QC], f32, tag="sd", name="sd")
                        if last and u == 1:
                            nc.scalar.copy(sa[:], pv[u][0:HD, :])
                            nc.scalar.copy(sd[:], pv[u][HD : HD + 1, :])
                        else:
                            nc.vector.tensor_copy(sa[:], pv[u][0:HD, :])
                            nc.vector.tensor_copy(sd[:], pv[u][HD : HD + 1, :])

                        def fin(u=u, sa=sa, sd=sd, hp=hp, j=j):
                            rd = nrsp.tile([1, QC], f32, tag="sd", name="rd")
                            nc.vector.reciprocal_approx_fast(rd[:], sd[:])
                            bc = nrmp.tile([HD, QC], f32, tag="sa", name="bc")
                            nc.gpsimd.partition_broadcast(bc[:], rd[:])
                            nc.vector.tensor_tensor(
                                out=AOT[hp][
                                    64 * u : 64 * u + 64, j * QC : (j + 1) * QC
                                ],
                                in0=sa[:],
                                in1=bc[:],
                                op=mybir.AluOpType.mult,
                            )

                        fins.append(fin)
                    # flush any leftover finishers of the previous chunk, then
                    # queue this chunk's
                    for fn in pending_fin:
                        fn()
                    pending_fin = fins
                    # after the last head-pair finishes chunk j, its tokens'
                    # output projection is unlocked
                    if hp == 3:
                        if not last:
                            for fn in pending_fin:
                                fn()
                            pending_fin = []
                            for tt in range(4 * j, 4 * j + 4):
                                for h in (0, 1):
                                    out_ready.append((tt, h))
                        else:
                            # tail: the held-back chains keep the PE busy
                            # while the final norm chain runs; evictions on
                            # scalar (no exps left).  Then the final chunk's
                            # chains, evictions alternating vector/scalar.
                            while out_ready:
                                out_chain(*out_ready.pop(0), evict=nc.scalar.copy)
                            for fn in pending_fin:
                                fn()
                            pending_fin = []
                            for tt in range(4 * j, 4 * j + 4):
                                for h in (0, 1):
                                    out_chain(
                                        tt, h,
                                        evict=nc.scalar.copy if h else None,
                                    )
                # drain any unconsumed fillers before the next head-pair
                for st_ in sorted(sched[hp]):
                    for fn in sched[hp][st_]:
                        fn()
                sched[hp] = {}
            for fn in pending_fin:
                fn()
            pending_fin = []
            # drain remaining out-proj chains (safety net; normally empty)
            while out_ready:
                out_chain(*out_ready.pop(0))
    nc.compile()
    return nc


def get_program(tok=T):
    if tok not in _prog_cache:
        _prog_cache[tok] = build_program(tok)
    return _prog_cache[tok]


def _pack_pmaj(a, nchunk):
    """[nchunk*128, F] -> [128, nchunk*F] partition-major."""
    F = a.shape[1]
    return np.ascontiguousarray(
        a.reshape(nchunk, 128, F).transpose(1, 0, 2).reshape(128, nchunk * F)
    )


def _hi_lo(a):
    """fp8e4 residual split: a ~= hi + lo with hi = fp8(a)."""
    f8 = ml_dtypes.float8_e4m3fn
    hi = a.astype(f8)
    lo = (a - hi.astype(np.float32)).astype(f8)
    return hi, lo


def make_in_maps(x, w_qkv, w_out):
    """Shard full inputs into 8 per-core input maps (fp8 hi/lo + bf16)."""
    bf = ml_dtypes.bfloat16
    x = np.asarray(x, dtype=np.float32)
    # wq/wk/wv host-scaled x16 so fp8 residuals stay in normal range;
    # folded back via the exp scale (1/256) and wo (1/16)
    w_qkv = np.asarray(w_qkv, dtype=np.float32) * 16.0
    w_out = np.asarray(w_out, dtype=np.float32) / 16.0
    w_out = w_out.astype(bf)
    D = D_MODEL
    # x[b].T partition-major [128, l, tok] then token-chunk-major pieces
    # xT{c} = [128, l, 512] for token chunk c, split fp8 hi/lo
    xTs = []
    for b in range(x.shape[0]):
        pm = _pack_pmaj(np.ascontiguousarray(x[b].T), 8)  # [128, 8*2048] f32
        pm = pm.reshape(128, 8, 4, 512)
        hi, lo = _hi_lo(pm)
        xTs.append(
            [
                (
                    np.ascontiguousarray(hi[:, :, c]).reshape(128, 8 * 512),
                    np.ascontiguousarray(lo[:, :, c]).reshape(128, 8 * 512),
                )
                for c in range(4)
            ]
        )
    in_maps = []
    for c in range(N_CORES):
        b, hg = c // 2, c % 2
        m = {}
        for cc in range(4):
            m[f"xTh{cc}"], m[f"xTl{cc}"] = xTs[b][cc]
        wq = _pack_pmaj(w_qkv[:, hg * FQ : (hg + 1) * FQ], 8)  # [128, l*512] f32
        wk = _pack_pmaj(w_qkv[:, D + hg * FQ : D + (hg + 1) * FQ], 8)
        wv = _pack_pmaj(w_qkv[:, 2 * D + hg * FQ : 2 * D + (hg + 1) * FQ], 8)
        for ft in range(4):
            for nm, w in (("wq", wq), ("wk", wk)):
                hi, lo = _hi_lo(
                    np.ascontiguousarray(
                        w.reshape(128, 8, 512)[:, :, ft * 128 : (ft + 1) * 128]
                    ).reshape(128, 8 * 128)
                )
                m[f"{nm}{ft}h"], m[f"{nm}{ft}l"] = hi, lo
        m["wvh"], m["wvl"] = _hi_lo(wv)
        wo = _pack_pmaj(
            w_out[hg * FQ : (hg + 1) * FQ, :].astype(np.float32), 4
        ).astype(bf)  # [128, d*1024]
        wor = wo.reshape(128, 4, 1024)
        m["wo0"] = np.ascontiguousarray(wor[:, :, :512]).reshape(128, 4 * 512)
        m["wo1"] = np.ascontiguousarray(wor[:, :, 512:]).reshape(128, 4 * 512)
        in_maps.append(m)
    return in_maps


_runner_cache = {}


def _make_runner(nc, n_cores=N_CORES):
    """Cached multi-core executor (same semantics as bass2jax.run_bass_via_pjrt
    for a program with no partition-id and no debug tensors, but the jitted
    callable is reusable so repeat kernel() calls don't recompile)."""
    import jax
    from jax.sharding import Mesh, PartitionSpec
    from jax.experimental.shard_map import shard_map
    import concourse.mybir as mybir
    from concourse.bass2jax import _bass_exec_p, install_neuronx_cc_hook

    install_neuronx_cc_hook()

    in_names, out_names, out_avals = [], [], []
    for alloc in nc.m.functions[0].allocations:
        if not isinstance(alloc, mybir.MemoryLocationSet):
            continue
        name = alloc.memorylocations[0].name
        if alloc.kind == "ExternalInput":
            in_names.append(name)
        elif alloc.kind == "ExternalOutput":
            out_names.append(name)
            out_avals.append(
                jax.core.ShapedArray(
                    tuple(alloc.tensor_shape), mybir.dt.np(alloc.dtype)
                )
            )
    n_params = len(in_names)
    n_outs = len(out_avals)
    all_in_names = in_names + out_names

    def _body(*args):
        outs = _bass_exec_p.bind(
            *args,
            out_avals=tuple(out_avals),
            in_names=tuple(all_in_names),
            out_names=tuple(out_names),
            lowering_input_output_aliases=(),
            sim_require_finite=True,
            sim_require_nnan=True,
            nc=nc,
        )
        return tuple(outs)

    devices = jax.devices()[:n_cores]
    mesh = Mesh(np.asarray(devices), ("core",))
    donate = tuple(range(n_params, n_params + n_outs))
    sharded = jax.jit(
        shard_map(
            _body,
            mesh=mesh,
            in_specs=(PartitionSpec("core"),) * (n_params + n_outs),
            out_specs=(PartitionSpec("core"),) * n_outs,
            check_rep=False,
        ),
        donate_argnums=donate,
        keep_unused=True,
    )

    def run(in_maps):
        per_core = [[np.asarray(m[nm]) for nm in in_names] for m in in_maps]
        concat_in = [
            np.concatenate([per_core[c][i] for c in range(n_cores)], axis=0)
            for i in range(n_params)
        ]
        concat_zeros = [
            np.zeros((n_cores * a.shape[0], *a.shape[1:]), a.dtype)
            for a in out_avals
        ]
        out_arrs = sharded(*concat_in, *concat_zeros)
        return [
            {
                nm: np.asarray(out_arrs[i]).reshape(n_cores, *out_avals[i].shape)[c]
                for i, nm in enumerate(out_names)
            }
            for c in range(n_cores)
        ]

    return run


def get_runner(tok=T):
    if tok not in _runner_cache:
        _runner_cache[tok] = _make_runner(get_program(tok))
    return _runner_cache[tok]


def kernel(x, w_qkv, w_out, b_out):
    in_maps = make_in_maps(x, w_qkv, w_out)
    try:
        run = get_runner(T)
        results = run(in_maps)
    except Exception:
        # fallback: the stock SPMD runner (recompiles per call but is the
        # battle-tested path)
        from concourse.bass_utils import run_bass_kernel_spmd

        results = run_bass_kernel_spmd(
            get_program(T), in_maps, list(range(N_CORES))
        ).results
    b_out = np.asarray(b_out, dtype=np.float32)
    out = np.empty((B, T, D_MODEL), dtype=np.float32)
    for b in range(B):
        out[b] = (
            results[2 * b]["y"].astype(np.float32)
            + results[2 * b + 1]["y"].astype(np.float32)
            + b_out
        )
    return out


# revision 22
# speedup vs baseline: 1.1474x; 1.1474x over previous
"""Causal self-attention TRN2 Bass kernel (bf16, software-pipelined).

Sharding: 8 cores = 4 batches x 2 head-groups. Core c handles batch c//2 and
heads (c%2)*8 .. (c%2)*8+8 (of 16). Each core computes its heads' attention
and a partial output projection; the host sums the two partials per batch and
adds b_out.

Design notes:
  - all matmul operands bf16 (FWL weight loads overlap the stream; f32r
    self-loading matmuls serialize a ~180ns weight load per matmul)
  - DRAM inputs are host-packed into small fine-grained tiles so the first
    S matmul only waits on ~1.5MB (xT is token-chunk-major, weights per-ft)
  - single pool scope, one long instruction stream: QK/V/out projections are
    emitted as deadline-scheduled "filler" half-chains interleaved into the
    attention cadence, so the PE never idles
  - causal mask applied by zeroing exp(S) tiles (affine_select on gpsimd/
    vector), not by adding -inf into PSUM: keeps DVE off the S->exp path
  - softmax denominator via the ones-column of V (row 64 of the PV PSUM);
    normalization split: PSUM drain (copies) immediate, recip/broadcast/mult
    deferred into later steps as filler DVE/gpsimd work
  - warmup matmuls ramp the PE p-state while input DMAs stream

Layouts on chip (per core):
  XTC   4 x [128, 8x512] bf16  x[b].T token-chunk-major: chunk c, d-chunk l
  WQF/WKF 4 x [128, 8x128] bf16 per-ft Q/K weights; WV 2 x [128, 4x512]
  WOH   2 x [128, 4x512] bf16
  QKT   8 x [128, 2048] bf16  Q^T (0..3) / K^T (4..7) features x tokens
  V     16 x [128, 520] bf16  tokens x (8 heads x (64 vals + ones col))
  e     [128, 1024] bf16      exp(S^T) per k-tile, both heads
  AOT   4 x [128, 2048] bf16  normalized attention out (features x tokens)
  y     [2048, 1024] bf16     partial output projection
"""
import sys

sys.path.insert(0, "/opt/trn_rl_repo")

import numpy as np
import ml_dtypes

D_MODEL = 1024
N_HEADS = 16
B = 4
T = 2048
HD = 64
N_CORES = 8
NH_LOC = N_HEADS // 2  # heads per core
FQ = NH_LOC * HD  # 512 local features

_prog_cache = {}


def build_program(tok=T, debug_dumps=False):
    """Build the single-core SPMD Bass program. tok must be a multiple of 512."""
    import concourse.mybir as mybir
    import concourse.tile as tile
    from concourse import bacc

    f32 = mybir.dt.float32
    bf16 = mybir.dt.bfloat16
    P = 128
    QC = 512  # q-chunk width
    KC = D_MODEL // P  # 8 d-model chunks
    TT = tok // P  # token tiles
    NJ = tok // QC  # q-chunks
    NDC = FQ // P  # 4 feature chunks

    nc = bacc.Bacc("TRN2", target_bir_lowering=False, debug=False, num_devices=N_CORES)

    # fine-grained DRAM inputs (host-packed); one tensor per DMA piece so
    # tile-granular dependencies stay small
    xTc = [
        nc.dram_tensor(f"xT{c}", [P, KC * QC], bf16, kind="ExternalInput")
        for c in range(NJ)
    ]
    wqf = [
        nc.dram_tensor(f"wq{ft}", [P, KC * P], bf16, kind="ExternalInput")
        for ft in range(NDC)
    ]
    wkf = [
        nc.dram_tensor(f"wk{ft}", [P, KC * P], bf16, kind="ExternalInput")
        for ft in range(NDC)
    ]
    wvh = [
        nc.dram_tensor(f"wv{h}", [P, 4 * FQ], bf16, kind="ExternalInput")
        for h in range(2)
    ]
    woh = [
        nc.dram_tensor(f"wo{h}", [P, NDC * QC], bf16, kind="ExternalInput")
        for h in range(2)
    ]
    y = nc.dram_tensor("y", [tok, D_MODEL], bf16, kind="ExternalOutput")

    with tile.TileContext(nc) as tc:
        with (
            tc.tile_pool(name="wp", bufs=1) as wp,
            tc.tile_pool(name="xtp", bufs=1) as xtp,
            tc.tile_pool(name="qktp", bufs=1) as qktp,
            tc.tile_pool(name="vp", bufs=1) as vp,
            tc.tile_pool(name="aotp", bufs=1) as aotp,
            tc.tile_pool(name="ep", bufs=3) as ep,
            tc.tile_pool(name="ystp", bufs=4) as ystp,
            tc.tile_pool(name="mvp", bufs=1) as mvp,
            tc.tile_pool(name="nrm", bufs=6) as nrmp,
            tc.tile_pool(name="nrs", bufs=6) as nrsp,
            tc.tile_pool(name="big", bufs=2, space="PSUM") as bigp,   # 4 banks
            tc.tile_pool(name="pvp", bufs=2, space="PSUM") as pvp,    # 2 banks
            tc.tile_pool(name="prj", bufs=2, space="PSUM") as prjp,   # 2 banks
        ):
            XTC = [wp.tile([P, KC * QC], bf16, tag=f"xtc{c}", name=f"xtc{c}") for c in range(NJ)]
            WQF = [wp.tile([P, KC * P], bf16, tag=f"wqf{ft}", name=f"wqf{ft}") for ft in range(NDC)]
            WKF = [wp.tile([P, KC * P], bf16, tag=f"wkf{ft}", name=f"wkf{ft}") for ft in range(NDC)]
            WVH = [wp.tile([P, 4 * FQ], bf16, tag=f"wvh{h}", name=f"wvh{h}") for h in range(2)]
            WOH = [wp.tile([P, NDC * QC], bf16, tag=f"woh{h}", name=f"woh{h}") for h in range(2)]
            QKT = [qktp.tile([P, tok], bf16, tag=f"qkt{i}", name=f"qkt{i}") for i in range(8)]
            V = [vp.tile([P, NH_LOC * (HD + 1)], bf16, tag=f"v{i}", name=f"v{i}") for i in range(TT)]
            AOT = [aotp.tile([P, tok], bf16, tag=f"aot{d}", name=f"aot{d}") for d in range(NDC)]

            # views
            def xt_view(l, c):  # [128, 512] of d-chunk l, token chunk c
                return XTC[c][:, l * QC : (l + 1) * QC]

            def wq_view(ft, l):
                return WQF[ft][:, l * P : (l + 1) * P]

            def wk_view(ft, l):
                return WKF[ft][:, l * P : (l + 1) * P]

            def wv_view(l):  # [128, 512] all 8 heads' V features, d-chunk l
                return WVH[l // 4][:, (l % 4) * FQ : (l % 4 + 1) * FQ]

            def wo_view(h, d):
                return WOH[h][:, d * QC : (d + 1) * QC]

            # ---------------- input DMAs ----------------
            # three issue queues in parallel, DMA-bandwidth-ordered: the
            # first-S critical set (xtc0 + wq_ft0 + wk_ft0, ~1.5MB) leads,
            # split so no queue serialises more than its share. The xtc
            # pieces are halved (l 0-3 / l 4-7) so region-granular deps let
            # half-chains start as soon as their half lands.
            HX = KC * QC // 2
            nc.gpsimd.dma_start(out=XTC[0][:, :HX], in_=xTc[0][:, :HX])
            nc.sync.dma_start(out=XTC[0][:, HX:], in_=xTc[0][:, HX:])
            nc.scalar.dma_start(out=WQF[0][:], in_=wqf[0][:])
            nc.scalar.dma_start(out=WKF[0][:], in_=wkf[0][:])
            nc.gpsimd.dma_start(out=WVH[0][:], in_=wvh[0][:])
            nc.sync.dma_start(out=WVH[1][:], in_=wvh[1][:])
            nc.gpsimd.dma_start(out=XTC[1][:, :HX], in_=xTc[1][:, :HX])
            nc.sync.dma_start(out=XTC[1][:, HX:], in_=xTc[1][:, HX:])
            nc.scalar.dma_start(out=WQF[1][:], in_=wqf[1][:])
            nc.scalar.dma_start(out=WKF[1][:], in_=wkf[1][:])
            nc.gpsimd.dma_start(out=XTC[2][:, :HX], in_=xTc[2][:, :HX])
            nc.sync.dma_start(out=XTC[2][:, HX:], in_=xTc[2][:, HX:])
            nc.gpsimd.dma_start(out=XTC[3][:, :HX], in_=xTc[3][:, :HX])
            nc.sync.dma_start(out=XTC[3][:, HX:], in_=xTc[3][:, HX:])
            nc.scalar.dma_start(out=WQF[2][:], in_=wqf[2][:])
            nc.scalar.dma_start(out=WKF[2][:], in_=wkf[2][:])
            nc.scalar.dma_start(out=WQF[3][:], in_=wqf[3][:])
            nc.scalar.dma_start(out=WKF[3][:], in_=wkf[3][:])
            nc.sync.dma_start(out=WOH[0][:], in_=woh[0][:])
            nc.sync.dma_start(out=WOH[1][:], in_=woh[1][:])

            # warm the exp table while input DMAs stream
            warm = mvp.tile([1, 8], f32, tag="warm", name="warm")
            nc.vector.memset(warm[:], 0.0)
            nc.scalar.activation(warm[:], warm[:], mybir.ActivationFunctionType.Exp)

            # ones columns of V (value cols are written by the projection
            # eviction; only col 64 of each head group needs initialising).
            # On the vector queue: gpsimd is busy issuing DMAs and must be
            # free early for the first diagonal ezeros.
            for tt in range(TT):
                vv = V[tt][:].rearrange("p (u c) -> p u c", c=HD + 1)
                nc.vector.memset(vv[:, :, HD : HD + 1], 1.0)

            # PE p-state warmup: dummy matmuls with no DMA deps keep the PE
            # "continuously executing" so real matmuls start at full clock
            dwa = mvp.tile([P, P], bf16, tag="dwa", name="dwa")
            dwb = mvp.tile([P, QC], bf16, tag="dwb", name="dwb")
            nc.vector.memset(dwa[:], 0.0)
            nc.vector.memset(dwb[:], 0.0)
            pwarm = prjp.tile([P, QC], f32, tag="prj", name="pwarm")
            for _ in range(16):
                nc.tensor.matmul(pwarm[:, :256], dwa[:], dwb[:, :256], start=True, stop=True)

            # ---------------- filler chains (emitted in halves) ----------------
            open_chains = {}

            def qk_half(ft, c, part):
                """QKT[ft][:, c-chunk] = (w-slice)^T @ XT over l; 2 halves."""
                wv_ = wq_view if ft < 4 else wk_view
                fi = ft % 4
                key = ("qk", ft, c)
                if part == 0:
                    open_chains[key] = prjp.tile([P, QC], f32, tag="prj", name=f"pqk{ft}_{c}")
                p = open_chains[key]
                for l in range(4 * part, 4 * part + 4):
                    nc.tensor.matmul(
                        p[:],
                        wv_(fi, l),
                        xt_view(l, c),
                        start=(l == 0),
                        stop=(l == KC - 1),
                    )
                if part == 1:
                    del open_chains[key]
                    nc.vector.tensor_copy(QKT[ft][:, c * QC : (c + 1) * QC], p[:])

            def v_half(tt, part):
                """V[tt] value cols = XT-slice^T @ WV; 2 halves."""
                key = ("v", tt)
                if part == 0:
                    open_chains[key] = prjp.tile([P, FQ], f32, tag="prj", name=f"pv{tt}")
                p = open_chains[key]
                c, s = tt // 4, tt % 4
                for l in range(4 * part, 4 * part + 4):
                    nc.tensor.matmul(
                        p[:],
                        xt_view(l, c)[:, s * P : (s + 1) * P],
                        wv_view(l),
                        start=(l == 0),
                        stop=(l == KC - 1),
                    )
                if part == 1:
                    del open_chains[key]
                    vdst = V[tt][:].rearrange("p (u c) -> p u c", c=HD + 1)[:, :, 0:HD]
                    vsrc = p[:].rearrange("p (u c) -> p u c", c=HD)
                    nc.vector.tensor_copy(vdst, vsrc)

            def out_chain(tt, h, evict=None):
                """y[tt-tile, h-half] = AOT-slice^T @ WO, 4 MMs + copy + DMA."""
                p = prjp.tile([P, QC], f32, tag="prj", name=f"py{tt}_{h}")
                for d in range(NDC):
                    nc.tensor.matmul(
                        p[:],
                        AOT[d][:, tt * P : (tt + 1) * P],
                        wo_view(h, d),
                        start=(d == 0),
                        stop=(d == NDC - 1),
                    )
                ysb = ystp.tile([P, QC], bf16, tag="y")
                if evict is None:
                    nc.vector.tensor_copy(ysb[:], p[:])
                else:
                    evict(ysb[:], p[:])
                nc.sync.dma_start(
                    out=y[tt * P : (tt + 1) * P, h * QC : (h + 1) * QC],
                    in_=ysb[:],
                )

            # per-hp filler schedules: {step: [unit, ...]}.  hp0's schedule is
            # deadline-driven (chunk c is read by attention from j=c onward at
            # step 4*c*(c+1)/2...; V[tt] is read by PV at the k-step for tile
            # tt of each j >= tt//4).
            sched = {hp: {} for hp in range(4)}

            def put(hp, step, fn):
                sched[hp].setdefault(step, []).append(fn)

            # hp0, deadline-driven: QK c1 at steps 0-3 (read from step 4),
            # V4-7 at 4-7 (read 8-11), QK c2 at 8-11 (read 12), V8-11 at
            # 12-15 (read 16-19), QK c3 at 16-19 (read 24), V12-15 at 20-23
            # (read 28-31)
            for k, (ft, c) in enumerate([(0, 1), (4, 1)]):
                for part in (0, 1):
                    put(0, 2 * k + part, lambda ft=ft, c=c, part=part: qk_half(ft, c, part))
            for tt in (4, 5, 6, 7):
                for part in (0, 1):
                    put(0, tt, lambda tt=tt, part=part: v_half(tt, part))
            for k, (ft, c) in enumerate([(0, 2), (4, 2)]):
                for part in (0, 1):
                    put(0, 8 + 2 * k + part, lambda ft=ft, c=c, part=part: qk_half(ft, c, part))
            for tt in (8, 9, 10, 11):
                for part in (0, 1):
                    put(0, 4 + tt, lambda tt=tt, part=part: v_half(tt, part))
            for k, (ft, c) in enumerate([(0, 3), (4, 3)]):
                for part in (0, 1):
                    put(0, 16 + 2 * k + part, lambda ft=ft, c=c, part=part: qk_half(ft, c, part))
            for tt in (12, 13, 14, 15):
                for part in (0, 1):
                    put(0, 8 + tt, lambda tt=tt, part=part: v_half(tt, part))
            # QK(hp+1) spread over each hp's steps (hp0's start after its
            # deadline units; hp1/hp2 evenly — ~1 unit per 2.5 steps keeps
            # the per-step PE load under the exp cadence)
            for hp in range(3):
                base = 24 if hp == 0 else 0
                units = []
                for ft in (hp + 1, 4 + hp + 1):
                    for c in range(NJ):
                        for part in (0, 1):
                            units.append(
                                lambda ft=ft, c=c, part=part: qk_half(ft, c, part)
                            )
                # finish by ~80% of the hp's steps so the next head-pair's
                # first S never waits on the last chain's eviction
                span = max(len(units), int(0.8 * (40 - base)))
                for k, fn in enumerate(units):
                    put(hp, base + (k * span) // len(units), fn)

            # ---------------- pre-attention work ----------------
            # half-chain interleave: the QK c0 chains (gating the first S)
            # complete as early as the DMA pieces allow, V chains fill in
            qk_half(0, 0, 0)
            qk_half(4, 0, 0)
            qk_half(0, 0, 1)
            qk_half(4, 0, 1)
            for tt in range(4):
                for part in (0, 1):
                    v_half(tt, part)

            # ---------------- attention + interleaved fillers ----------------
            out_ready = []  # out-proj chains unlocked so far
            pending_fin = []  # deferred normalization finishers
            for hp in range(4):
                step = 0
                for j in range(NJ):
                    nkt = 4 * j + 4
                    pv = {
                        u: pvp.tile([HD + 1, QC], f32, tag="pv", name=f"pv{u}")
                        for u in (0, 1)
                    }
                    etiles = {}

                    def emit_s_exp(i, j=j, etiles=etiles):
                        """S-pair + exp (+ causal zeroing) for k-tile i."""
                        s_ = i - 4 * j
                        w0 = 128 * s_ if s_ >= 0 else 0
                        st = bigp.tile([P, 2 * QC], f32, tag="big", name="st")
                        for u in (0, 1):
                            rs = slice(64 * u, 64 * u + 64)
                            nc.tensor.matmul(
                                st[:, u * QC + w0 : (u + 1) * QC],
                                QKT[4 + hp][rs, i * P : (i + 1) * P],
                                QKT[hp][rs, j * QC + w0 : (j + 1) * QC],
                                start=True,
                                stop=True,
                            )
                        win3 = st[:].rearrange("p (h q) -> p h q", h=2)
                        e = ep.tile([P, 2 * QC], bf16, tag="e", name="e")
                        nc.scalar.activation(
                            e[:].rearrange("p (h q) -> p h q", h=2)[:, :, w0:QC],
                            win3[:, :, w0:QC],
                            mybir.ActivationFunctionType.Exp,
                            scale=0.125,
                        )
                        if s_ >= 0:
                            # causal mask: zero exp(S) where q < k in the
                            # diagonal 128x128 block (iota = q_hat - p >= 0
                            # keeps; else fill 0).  gpsimd/vector split.
                            for u, eng in ((0, nc.gpsimd), (1, nc.gpsimd)):
                                ev = e[:, u * QC + w0 : u * QC + w0 + P]
                                eng.affine_select(
                                    out=ev,
                                    in_=ev,
                                    compare_op=mybir.AluOpType.is_ge,
                                    fill=0.0,
                                    base=0,
                                    pattern=[[1, P]],
                                    channel_multiplier=-1,
                                )
                        etiles[i] = e

                    emit_s_exp(0)
                    if nkt > 1:
                        emit_s_exp(1)
                    for i in range(nkt):
                        # software pipeline, lookahead 2: issue S/exp two
                        # k-tiles ahead of this PV so the PE's in-order queue
                        # always has an S-pair between consecutive PVs and the
                        # ACT stream gets a full step of slack
                        if i + 2 < nkt:
                            emit_s_exp(i + 2)
                        s_ = i - 4 * j
                        w0 = 128 * s_ if s_ >= 0 else 0
                        e = etiles.pop(i)
                        for u in (0, 1):
                            hloc = 2 * hp + u
                            nc.tensor.matmul(
                                pv[u][:, w0:QC],
                                V[i][:, hloc * (HD + 1) : (hloc + 1) * (HD + 1)],
                                e[:, u * QC + w0 : (u + 1) * QC],
                                start=(i == 0),
                                stop=(i == nkt - 1),
                            )
                        # deferred norm finishers go first (they unlock AOT
                        # for out chains), on non-diagonal steps only so the
                        # gpsimd broadcast never delays a diagonal ezero
                        if pending_fin and s_ < 0:
                            pending_fin.pop(0)()
                        for fn in sched[hp].pop(step, ()):
                            fn()
                        step += 1
                        # out-proj chains become fillers once unlocked; in the
                        # last head-pair keep 4 in reserve so the PE has work
                        # to chew while the final chunk's normalization runs
                        if out_ready and len(out_ready) > 4:
                            out_chain(*out_ready.pop(0))
                    # normalization, split in two: drain PSUM now (copies of
                    # the accumulator rows and the denominator row), defer
                    # recip/broadcast/mult into later steps.  For the very
                    # last chunk, u1's copies go to the (now exp-free) scalar
                    # engine so the two PSUM drains run in parallel.
                    last = hp == 3 and j == NJ - 1
                    fins = []
                    for u in (0, 1):
                        sa = nrmp.tile([HD, QC], f32, tag="sa", name="sa")
                        sd = nrsp.tile([1, QC], f32, tag="sd", name="sd")
                        if last and u == 1:
                            nc.scalar.copy(sa[:], pv[u][0:HD, :])
                            nc.scalar.copy(sd[:], pv[u][HD : HD + 1, :])
                        else:
                            nc.vector.tensor_copy(sa[:], pv[u][0:HD, :])
                            nc.vector.tensor_copy(sd[:], pv[u][HD : HD + 1, :])

                        def fin(u=u, sa=sa, sd=sd, hp=hp, j=j):
                            rd = nrsp.tile([1, QC], f32, tag="sd", name="rd")
                            nc.vector.reciprocal_approx_fast(rd[:], sd[:])
                            bc = nrmp.tile([HD, QC], f32, tag="sa", name="bc")
                            nc.gpsimd.partition_broadcast(bc[:], rd[:])
                            nc.vector.tensor_tensor(
                                out=AOT[hp][
                                    64 * u : 64 * u + 64, j * QC : (j + 1) * QC
                                ],
                                in0=sa[:],
                                in1=bc[:],
                                op=mybir.AluOpType.mult,
                            )

                        fins.append(fin)
                    # flush any leftover finishers of the previous chunk, then
                    # queue this chunk's
                    for fn in pending_fin:
                        fn()
                    pending_fin = fins
                    # after the last head-pair finishes chunk j, its tokens'
                    # output projection is unlocked
                    if hp == 3:
                        if not last:
                            for fn in pending_fin:
                                fn()
                            pending_fin = []
                            for tt in range(4 * j, 4 * j + 4):
                                for h in (0, 1):
                                    out_ready.append((tt, h))
                        else:
                            # tail: the held-back chains keep the PE busy
                            # while the final norm chain runs; evictions on
                            # scalar (no exps left).  Then the final chunk's
                            # chains, evictions alternating vector/scalar.
                            while out_ready:
                                out_chain(*out_ready.pop(0), evict=nc.scalar.copy)
                            for fn in pending_fin:
                                fn()
                            pending_fin = []
                            for tt in range(4 * j, 4 * j + 4):
                                for h in (0, 1):
                                    out_chain(
                                        tt, h,
                                        evict=nc.scalar.copy if h else None,
                                    )
                # drain any unconsumed fillers before the next head-pair
                for st_ in sorted(sched[hp]):
                    for fn in sched[hp][st_]:
                        fn()
                sched[hp] = {}
            for fn in pending_fin:
                fn()
            pending_fin = []
            # drain remaining out-proj chains (safety net; normally empty)
            while out_ready:
                out_chain(*out_ready.pop(0))
    nc.compile()
    return nc


def get_program(tok=T):
    if tok not in _prog_cache:
        _prog_cache[tok] = build_program(tok)
    return _prog_cache[tok]


def _pack_pmaj(a, nchunk):
    """[nchunk*128, F] -> [128, nchunk*F] partition-major."""
    F = a.shape[1]
    return np.ascontiguousarray(
        a.reshape(nchunk, 128, F).transpose(1, 0, 2).reshape(128, nchunk * F)
    )


def make_in_maps(x, w_qkv, w_out):
    """Shard full inputs into 8 per-core input maps (bf16, packed layouts)."""
    bf = ml_dtypes.bfloat16
    x = np.asarray(x, dtype=np.float32)
    w_qkv = np.asarray(w_qkv, dtype=np.float32).astype(bf)
    w_out = np.asarray(w_out, dtype=np.float32).astype(bf)
    D = D_MODEL
    # x[b].T partition-major [128, l, tok] then token-chunk-major pieces
    # xT{c} = [128, l, 512] for token chunk c
    xTs = []
    for b in range(x.shape[0]):
        pm = _pack_pmaj(np.ascontiguousarray(x[b].T).astype(bf), 8)  # [128, 8*2048]
        pm = pm.reshape(128, 8, 4, 512)
        xTs.append(
            [np.ascontiguousarray(pm[:, :, c]).reshape(128, 8 * 512) for c in range(4)]
        )
    in_maps = []
    for c in range(N_CORES):
        b, hg = c // 2, c % 2
        m = {}
        for cc in range(4):
            m[f"xT{cc}"] = xTs[b][cc]
        wq = _pack_pmaj(w_qkv[:, hg * FQ : (hg + 1) * FQ], 8)  # [128, l, 512]
        wk = _pack_pmaj(w_qkv[:, D + hg * FQ : D + (hg + 1) * FQ], 8)
        wv = _pack_pmaj(w_qkv[:, 2 * D + hg * FQ : 2 * D + (hg + 1) * FQ], 8)
        for ft in range(4):
            m[f"wq{ft}"] = np.ascontiguousarray(
                wq.reshape(128, 8, 512)[:, :, ft * 128 : (ft + 1) * 128]
            ).reshape(128, 8 * 128)
            m[f"wk{ft}"] = np.ascontiguousarray(
                wk.reshape(128, 8, 512)[:, :, ft * 128 : (ft + 1) * 128]
            ).reshape(128, 8 * 128)
        m["wv0"] = np.ascontiguousarray(wv[:, : 4 * 512])
        m["wv1"] = np.ascontiguousarray(wv[:, 4 * 512 :])
        wo = _pack_pmaj(w_out[hg * FQ : (hg + 1) * FQ, :], 4)  # [128, d, 1024]
        wor = wo.reshape(128, 4, 1024)
        m["wo0"] = np.ascontiguousarray(wor[:, :, :512]).reshape(128, 4 * 512)
        m["wo1"] = np.ascontiguousarray(wor[:, :, 512:]).reshape(128, 4 * 512)
        in_maps.append(m)
    return in_maps


_runner_cache = {}


def _make_runner(nc, n_cores=N_CORES):
    """Cached multi-core executor (same semantics as bass2jax.run_bass_via_pjrt
    for a program with no partition-id and no debug tensors, but the jitted
    callable is reusable so repeat kernel() calls don't recompile)."""
    import jax
    from jax.sharding import Mesh, PartitionSpec
    from jax.experimental.shard_map import shard_map
    import concourse.mybir as mybir
    from concourse.bass2jax import _bass_exec_p, install_neuronx_cc_hook

    install_neuronx_cc_hook()

    in_names, out_names, out_avals = [], [], []
    for alloc in nc.m.functions[0].allocations:
        if not isinstance(alloc, mybir.MemoryLocationSet):
            continue
        name = alloc.memorylocations[0].name
        if alloc.kind == "ExternalInput":
            in_names.append(name)
        elif alloc.kind == "ExternalOutput":
            out_names.append(name)
            out_avals.append(
                jax.core.ShapedArray(
                    tuple(alloc.tensor_shape), mybir.dt.np(alloc.dtype)
                )
            )
    n_params = len(in_names)
    n_outs = len(out_avals)
    all_in_names = in_names + out_names

    def _body(*args):
        outs = _bass_exec_p.bind(
            *args,
            out_avals=tuple(out_avals),
            in_names=tuple(all_in_names),
            out_names=tuple(out_names),
            lowering_input_output_aliases=(),
            sim_require_finite=True,
            sim_require_nnan=True,
            nc=nc,
        )
        return tuple(outs)

    devices = jax.devices()[:n_cores]
    mesh = Mesh(np.asarray(devices), ("core",))
    donate = tuple(range(n_params, n_params + n_outs))
    sharded = jax.jit(
        shard_map(
            _body,
            mesh=mesh,
            in_specs=(PartitionSpec("core"),) * (n_params + n_outs),
            out_specs=(PartitionSpec("core"),) * n_outs,
            check_rep=False,
        ),
        donate_argnums=donate,
        keep_unused=True,
    )

    def run(in_maps):
        per_core = [[np.asarray(m[nm]) for nm in in_names] for m in in_maps]
        concat_in = [
            np.concatenate([per_core[c][i] for c in range(n_cores)], axis=0)
            for i in range(n_params)
        ]
        concat_zeros = [
            np.zeros((n_cores * a.shape[0], *a.shape[1:]), a.dtype)
            for a in out_avals
        ]
        out_arrs = sharded(*concat_in, *concat_zeros)
        return [
            {
                nm: np.asarray(out_arrs[i]).reshape(n_cores, *out_avals[i].shape)[c]
                for i, nm in enumerate(out_names)
            }
            for c in range(n_cores)
        ]

    return run


def get_runner(tok=T):
    if tok not in _runner_cache:
        _runner_cache[tok] = _make_runner(get_program(tok))
    return _runner_cache[tok]


def kernel(x, w_qkv, w_out, b_out):
    in_maps = make_in_maps(x, w_qkv, w_out)
    try:
        run = get_runner(T)
        results = run(in_maps)
    except Exception:
        # fallback: the stock SPMD runner (recompiles per call but is the
        # battle-tested path)
        from concourse.bass_utils import run_bass_kernel_spmd

        results = run_bass_kernel_spmd(
            get_program(T), in_maps, list(range(N_CORES))
        ).results
    b_out = np.asarray(b_out, dtype=np.float32)
    out = np.empty((B, T, D_MODEL), dtype=np.float32)
    for b in range(B):
        out[b] = (
            results[2 * b]["y"].astype(np.float32)
            + results[2 * b + 1]["y"].astype(np.float32)
            + b_out
        )
    return out
